# revision 1
# baseline (speedup 1.0000x reference)
"""BiLevelRoutingAttention TRN2 Bass kernel — full-input entry point.

Strategy: data-parallel over batch (16 images -> 8 NeuronCores x 2).
Each core runs an identical Bass/Tile kernel over its 2 images:
  - router: window mean-pool (DVE reduces) + tiny GEMM + top-4 via vector.max
  - qkv GEMMs in bf16; V produced directly transposed ([token, ch]) so the
    attention AV contraction needs no separate V transpose
  - per (t, head-group) QK^T as one [128]x[128,196] matmul against a
    block-diagonal K tile (block-diag builds batch over t on GPSIMD)
  - masked softmax fused into the PSUM drain (scalar_tensor_tensor) + ACT exp
  - A^T via identity-matmul transposes into a 2-t block-diagonal PSUM layout;
    AV then contracts full 128 partitions against V^T directly
  - output projection in bf16, raster reorder fused into drain APs
No collectives are needed (pure batch parallelism).
"""
import numpy as np

import concourse.bass as bass
import concourse.tile as tile
import concourse.mybir as mybir
from concourse import bacc
from concourse.bass_utils import run_bass_kernel_spmd
from concourse.masks import make_identity

F32 = mybir.dt.float32
BF16 = mybir.dt.bfloat16
AL = mybir.AluOpType
ACTF = mybir.ActivationFunctionType
AX = mybir.AxisListType

N_CORES = 8
B_FULL = 16
NB = B_FULL // N_CORES       # batches per core
C = 512
H = W = 56
HW = 3136
NWIN_SIDE = 7
NW = 49
HEADS = 16
NEG = 1.0e9
SCALE = 32.0 ** -0.5
TB = 8


def _x_qkview(x_tile, th):
    """x3 is (th, tw, win)-major: th-block is contiguous [128, 392]."""
    return x_tile[:, th * 392:(th + 1) * 392]


def _x_tpair(x_tile, th, twp):
    """lhsT [128, 98=(tw-pair, 49win)]: contiguous slice of x3."""
    return x_tile[:, th * 392 + twp * 98:th * 392 + (twp + 1) * 98]


def _attn_tblock(nc, th, par, qblk, kblk, kbd_tiles, a_tiles, mb_t,
                 iden, vt_tiles, at_ps, at_sbs, o_tiles, ps_l, ps_o, sp):
    for c in range(4):
        kt = kbd_tiles[c][par]
        for j in range(4):
            src = kblk[c][32 * j:32 * (j + 1), :].rearrange(
                "p (t w) -> p t w", t=TB)
            dstv = kt[32 * j:32 * (j + 1), :].rearrange(
                "p (t g) -> p t g", t=TB)[:, :, 49 * j:49 * (j + 1)]
            nc.gpsimd.tensor_copy(dstv, src)
    a_hp = [a_tiles[hp][par] for hp in range(2)]
    for ti in range(TB):
        for hp in range(2):
            psl = ps_l.tile([NW, 392], F32, tag="ps_L",
                            name=f"psl{ti}_{hp}")
            for g in range(2):
                hg = hp * 2 + g
                lhsT = qblk[hg][:, ti * NW:(ti + 1) * NW]
                nc.tensor.matmul(
                    psl[:, g * 196:(g + 1) * 196], lhsT,
                    kbd_tiles[hg][par][:, ti * 196:(ti + 1) * 196],
                    start=True, stop=True, skip_group_check=True)
            nc.vector.scalar_tensor_tensor(
                out=a_hp[hp][:, ti * 392:(ti + 1) * 392],
                in0=psl[:], scalar=SCALE,
                in1=mb_t[:, hp * 392:(hp + 1) * 392],
                op0=AL.mult, op1=AL.add)
    for hp in range(2):
        nc.scalar.activation(a_hp[hp][:], a_hp[hp][:], ACTF.Exp)
        den = sp.tile([NW, TB * 8], F32, tag=f"den{hp}",
                      name=f"den{hp}")
        av = a_hp[hp][:].rearrange("q (a k) -> q a k", k=NW)
        nc.vector.reduce_sum(den[:], av, axis=AX.X)
        nc.vector.reciprocal(den[:], den[:])
        nc.vector.tensor_tensor(
            out=av, in0=av,
            in1=den[:].unsqueeze(2).broadcast_to([NW, TB * 8, NW]),
            op=AL.mult)
    for taui in range(TB // 2):
        tau = th * 4 + taui
        for quarter in range(4):
            hg = quarter
            pp = (taui * 4 + quarter) % 2
            colb = pp * 512
            for j in range(4):
                h = hg * 4 + j
                hp, g = h // 8, (h // 4) % 2
                base = (2 * taui) * 392 + g * 196 + j * 49
                base_o = (2 * taui + 1) * 392 + g * 196 + j * 49
                nc.tensor.matmul(
                    at_ps[0:49, colb + j * 128:colb + j * 128 + 49],
                    a_hp[hp][:, base:base + 49], iden[:],
                    start=True, stop=True, skip_group_check=True)
                nc.tensor.matmul(
                    at_ps[64:113,
                          colb + j * 128 + 49:colb + j * 128 + 98],
                    a_hp[hp][:, base_o:base_o + 49], iden[:],
                    start=True, stop=True, skip_group_check=True)
            at_sb = at_sbs[pp]
            nc.scalar.copy(
                at_sb[:],
                at_ps[:, colb:colb + 512].rearrange(
                    "p (r c) -> p r c", r=4)[:, :, 0:98])
            pso = ps_o.tile([128, 98], F32, tag="ps_o",
                            name=f"pso{quarter}")
            for j in range(4):
                h = hg * 4 + j
                nc.tensor.matmul(
                    pso[32 * j:32 * (j + 1), :],
                    vt_tiles[tau][:, h * 32:(h + 1) * 32],
                    at_sb[:, j * 98:(j + 1) * 98],
                    start=True, stop=True, skip_group_check=True,
                    tile_position=(0, 32 * j))
            dst = o_tiles[hg][:].rearrange(
                "p (w t) -> p t w",
                w=NW)[:, 2 * tau:2 * tau + 2, :]
            nc.scalar.copy(dst, pso[:])



def build_nc(nb=NB):
    nc = bacc.Bacc(None, target_bir_lowering=False, debug=False)

    xd = nc.dram_tensor("x", [nb, C, HW], F32, kind="ExternalInput")
    qkvw = nc.dram_tensor("qkv_wT", [C, 3 * C], F32, kind="ExternalInput")
    rw = nc.dram_tensor("r_wT", [C, NW], F32, kind="ExternalInput")
    rb = nc.dram_tensor("router_b", [1, NW], F32, kind="ExternalInput")
    pw = nc.dram_tensor("proj_wT", [C, C], F32, kind="ExternalInput")
    pb = nc.dram_tensor("proj_b", [1, C], F32, kind="ExternalInput")
    biasq = nc.dram_tensor("bias_q", [NW, HEADS * NW], F32, kind="ExternalInput")
    yd = nc.dram_tensor("y", [nb, C, HW], F32, kind="ExternalOutput")

    from contextlib import ExitStack
    with tile.TileContext(nc) as tc, ExitStack() as ctx:
        wp = ctx.enter_context(tc.tile_pool(name="weights", bufs=1))
        xp = ctx.enter_context(tc.tile_pool(name="xin", bufs=1))
        qkp = ctx.enter_context(tc.tile_pool(name="qk", bufs=1))
        vtp = ctx.enter_context(tc.tile_pool(name="vt", bufs=1))
        kbp = ctx.enter_context(tc.tile_pool(name="kbd", bufs=1))
        ap_ = ctx.enter_context(tc.tile_pool(name="attn", bufs=1))
        atp = ctx.enter_context(tc.tile_pool(name="atsb", bufs=2))
        op_ = ctx.enter_context(tc.tile_pool(name="oT", bufs=1))
        sp = ctx.enter_context(tc.tile_pool(name="small", bufs=1))
        yp = ctx.enter_context(tc.tile_pool(name="yst", bufs=2))
        ps_g = ctx.enter_context(tc.tile_pool(name="psg", bufs=2, space="PSUM"))
        ps_l = ctx.enter_context(tc.tile_pool(name="psl", bufs=2, space="PSUM"))
        ps_at = ctx.enter_context(tc.tile_pool(name="psat", bufs=1, space="PSUM"))
        ps_o = ctx.enter_context(tc.tile_pool(name="pso", bufs=2, space="PSUM"))

        # ---- persistent weights ----
        qkvw_t = wp.tile([128, 4, 3 * C], BF16, tag="qkvw")
        for cc in range(4):
            qwtmp = sp.tile([128, 3 * C], F32, tag="pwtmp", name=f"qwtmp{cc}")
            nc.sync.dma_start(qwtmp[:], qkvw[cc * 128:(cc + 1) * 128, :])
            nc.vector.tensor_copy(qkvw_t[:, cc, :], qwtmp[:])
        rw_t = wp.tile([128, 4, NW], F32, tag="rw")
        nc.sync.dma_start(rw_t[:], rw[:].rearrange("(a p) k -> p a k", p=128))
        rb_t = wp.tile([1, NW], F32, tag="rb")
        nc.sync.dma_start(rb_t[:], rb[:])
        rb64 = wp.tile([1, NW], BF16, tag="rb64")
        nc.vector.tensor_scalar_mul(rb64[:], rb_t[:], 64.0)
        pbp = wp.tile([128, 4], F32, tag="pbp")
        nc.sync.dma_start(pbp[:], pb[:].rearrange("o (a p) -> (o p) a", p=128))
        bq_t = wp.tile([NW, HEADS * NW], BF16, tag="bq")
        bqtmp = sp.tile([NW, HEADS * NW], F32, tag="bqtmp")
        nc.sync.dma_start(bqtmp[:], biasq[:])
        nc.vector.tensor_copy(bq_t[:], bqtmp[:])
        iden = wp.tile([NW, NW], BF16, tag="iden")
        make_identity(nc, iden[:])
        ones1 = wp.tile([1, NW], BF16, tag="ones1")
        nc.vector.memset(ones1[:], 1.0)
        pw_b = wp.tile([128, 4, C], BF16, tag="pwb")
        for cc in range(4):
            pwtmp = sp.tile([128, C], F32, tag="pwtmp", name=f"pwtmp{cc}")
            nc.sync.dma_start(pwtmp[:], pw[cc * 128:(cc + 1) * 128, :])
            nc.vector.tensor_copy(pw_b[:, cc, :], pwtmp[:])

        # ---- persistent zero-padded tiles ----
        vt_tiles = [vtp.tile([128, C], BF16, tag=f"vt{tp}", name=f"vt{tp}")
                    for tp in range(32)]
        for tp in range(32):
            nc.gpsimd.memset(vt_tiles[tp][:], 0.0)
        kbd_tiles = [[kbp.tile([128, TB * 196], BF16, tag=f"kbd{c}_{p}",
                               name=f"kbd{c}_{p}")
                      for p in range(2)] for c in range(4)]
        for c in range(4):
            for p in range(2):
                nc.gpsimd.memset(kbd_tiles[c][p][:], 0.0)

        x_tiles = [xp.tile([128, HW], BF16, tag=f"x{c}", name=f"x{c}")
                   for c in range(4)]
        o_tiles = [op_.tile([128, HW], BF16, tag=f"o{c}", name=f"ot{c}")
                   for c in range(4)]
        mb_t = sp.tile([NW, HEADS * NW], BF16, tag="mb")
        a_tiles = [[ap_.tile([NW, TB * 392], BF16, tag=f"a{hp}_{p}",
                             name=f"a{hp}_{p}")
                    for p in range(2)] for hp in range(2)]
        at_ps = ps_at.tile([128, 8 * 128], F32, tag="atps")
        nc.vector.memset(at_ps[:], 0.0)
        at_sbs = [atp.tile([128, 4 * 98], BF16, tag=f"at_sb{p}",
                           name=f"at_sb{p}") for p in range(2)]

        for b in range(nb):
            # ---- load x (f32 staging -> bf16) + router pooling ----
            xp_t = sp.tile([128, 4, NW], F32, tag="xpool")
            for c in range(4):
                xstage = xp.tile([128, HW], F32, tag="xstage", name=f"xs{c}")
                nc.sync.dma_start(xstage[:], xd[b, c * 128:(c + 1) * 128, :])
                xsv = xstage[:].rearrange("p (a t b u) -> p t u a b",
                                          a=7, t=8, b=7, u=8)
                x3v = x_tiles[c][:].rearrange("p (t u a b) -> p t u a b",
                                              t=8, u=8, a=7, b=7)
                nc.vector.tensor_copy(x3v, xsv)
                s1 = sp.tile([128, 56, 7], F32, tag="pool1")
                v = xstage[:].rearrange("p (h b u) -> p h b u", h=56, b=7, u=8)
                nc.vector.reduce_sum(s1[:], v, axis=AX.X)
                v2 = s1[:].rearrange("p (a t) b -> p a b t", a=7, t=8)
                nc.vector.reduce_sum(xp_t[:, c, :], v2, axis=AX.X)

            # ---- router scores + top-4 mask + mask/bias tile ----
            ps_s = ps_l.tile([NW, NW], F32, tag="ps_L")
            for c in range(4):
                nc.tensor.matmul(ps_s[:], xp_t[:, c, :], rw_t[:, c, :],
                                 start=(c == 0), stop=False)
            nc.tensor.matmul(ps_s[:], ones1[:], rb64[:], start=False, stop=True)
            s_sb = sp.tile([NW, NW], F32, tag="s_sb")
            nc.scalar.activation(s_sb[:], ps_s[:], ACTF.Copy, scale=1.0 / 64.0)
            t8 = sp.tile([NW, 8], F32, tag="t8")
            nc.vector.max(t8[:], s_sb[:])
            mask = sp.tile([NW, NW], F32, tag="mask")
            nc.vector.tensor_scalar(out=mask[:], in0=s_sb[:], scalar1=t8[:, 3:4],
                                    scalar2=None, op0=AL.is_ge)
            mneg = sp.tile([NW, NW], F32, tag="mneg")
            nc.vector.tensor_scalar(out=mneg[:], in0=mask[:], scalar1=NEG,
                                    scalar2=NEG, op0=AL.mult, op1=AL.subtract)
            nc.vector.tensor_tensor(
                out=mb_t[:].rearrange("q (h k) -> q h k", h=HEADS),
                in0=bq_t[:].rearrange("q (h k) -> q h k", h=HEADS),
                in1=mneg[:].unsqueeze(1).broadcast_to([NW, HEADS, NW]),
                op=AL.add)

            # ---- per t-block-pair: qkv (stationary reused) + attention ----
            for thp in range(TB // 2):
                th0 = thp * 2
                qkblk = [[], []]
                for m in range(8):
                    pss = [ps_g.tile([128, C], F32, tag="g",
                                     name=f"psqk{m}_{e}") for e in range(2)]
                    for c in range(4):
                        for e in range(2):
                            nc.tensor.matmul(
                                pss[e][:, 0:392],
                                qkvw_t[:, c, m * 128:(m + 1) * 128],
                                _x_qkview(x_tiles[c], th0 + e),
                                start=(c == 0), stop=(c == 3))
                    for e in range(2):
                        blk = qkp.tile([128, 392], BF16, tag=f"qk{m}_{e}",
                                       name=f"qk{m}_{e}")
                        nc.scalar.copy(blk[:], pss[e][:, 0:392])
                        qkblk[e].append(blk)
                for e in range(2):
                    th = th0 + e
                    for twp in range(4):
                        tau = th * 4 + twp
                        ps = ps_g.tile([128, C], F32, tag="g",
                                       name=f"psv{twp}")
                        for c in range(4):
                            nc.tensor.matmul(ps[0:98, :],
                                             _x_tpair(x_tiles[c], th, twp),
                                             qkvw_t[:, c, 2 * C:3 * C],
                                             start=(c == 0), stop=(c == 3))
                        vtmp = sp.tile([98, C], BF16, tag="vtmp",
                                       name=f"vtmp{twp}")
                        nc.scalar.copy(vtmp[:], ps[0:98, :])
                        nc.sync.dma_start(vt_tiles[tau][0:49, :], vtmp[0:49, :])
                        nc.sync.dma_start(vt_tiles[tau][64:113, :],
                                          vtmp[49:98, :])
                # ---- attention for each th in the pair ----
                for e in range(2):
                    th = th0 + e
                    par = e
                    qblk = qkblk[e][0:4]
                    kblk = qkblk[e][4:8]
                    _attn_tblock(
                        nc, th, par, qblk, kblk, kbd_tiles, a_tiles, mb_t,
                        iden, vt_tiles, at_ps, at_sbs, o_tiles,
                        ps_l, ps_o, sp)

            # ---- output projection ----
            # ---- output projection ----
            for mo in range(4):
                for nt in range(7):
                    ps = ps_g.tile([128, C], F32, tag="g", name=f"psy{nt}")
                    for c in range(4):
                        nc.tensor.matmul(
                            ps[:, 0:448], pw_b[:, c, mo * 128:(mo + 1) * 128],
                            o_tiles[c][:, nt * 448:(nt + 1) * 448],
                            start=(c == 0), stop=(c == 3))
                    yst = yp.tile([128, 448], F32, tag="yst", name=f"yst{nt}")
                    yv = yst[:].rearrange("p (t b u) -> p b t u", t=8, b=7)
                    nc.scalar.activation(
                        yv, ps[:, 0:448].rearrange("p (b t u) -> p b t u",
                                                   b=7, t=8),
                        ACTF.Identity, bias=pbp[:, mo:mo + 1])
                    nc.sync.dma_start(
                        yd[b, mo * 128:(mo + 1) * 128,
                           nt * 448:(nt + 1) * 448], yst[:])

    nc.compile()
    return nc


def _rel_index(n):
    coords = np.stack(np.meshgrid(np.arange(n), np.arange(n), indexing="ij"),
                      0).reshape(2, -1)
    rel = (coords[:, :, None] - coords[:, None, :]).transpose(1, 2, 0)
    rel[..., 0] += n - 1
    rel[..., 1] += n - 1
    rel[..., 0] *= 2 * n - 1
    return rel.sum(-1)


def host_prep(x, router_w, router_b, qkv_w, proj_w, proj_b, rpb_table):
    """Shared (per-core-identical) weight tensors + per-core x slices."""
    x = np.ascontiguousarray(np.asarray(x, np.float32).reshape(B_FULL, C, HW))
    rel = _rel_index(NWIN_SIDE)
    bias_q = np.asarray(rpb_table, np.float32)[rel]          # (49, 49, 16)
    bias_q = np.ascontiguousarray(bias_q.transpose(0, 2, 1)).reshape(NW,
                                                                     HEADS * NW)
    shared = {
        "qkv_wT": np.ascontiguousarray(np.asarray(qkv_w, np.float32).T),
        "r_wT": np.ascontiguousarray(np.asarray(router_w, np.float32).T),
        "router_b": np.ascontiguousarray(
            np.asarray(router_b, np.float32).reshape(1, NW)),
        "proj_wT": np.ascontiguousarray(np.asarray(proj_w, np.float32).T),
        "proj_b": np.ascontiguousarray(
            np.asarray(proj_b, np.float32).reshape(1, C)),
        "bias_q": bias_q,
    }
    in_maps = []
    for core in range(N_CORES):
        m = dict(shared)
        m["x"] = np.ascontiguousarray(x[core * NB:(core + 1) * NB])
        in_maps.append(m)
    return in_maps


_NC_CACHE = {}


def _get_nc():
    if "nc" not in _NC_CACHE:
        _NC_CACHE["nc"] = build_nc(NB)
    return _NC_CACHE["nc"]


def kernel(x, router_w, router_b, qkv_w, proj_w, proj_b, rpb_table):
    in_maps = host_prep(x, router_w, router_b, qkv_w, proj_w, proj_b, rpb_table)
    nc = _get_nc()
    res = run_bass_kernel_spmd(nc, in_maps, core_ids=list(range(N_CORES)))
    ys = [res.results[i]["y"] for i in range(N_CORES)]
    y = np.concatenate(ys, axis=0).reshape(B_FULL, C, H, W)
    return y.astype(np.float32)

